# revision 1
# baseline (speedup 1.0000x reference)
"""HCLT probabilistic-circuit kernel for 8 Trainium2 NeuronCores.

Math: the reference collapses algebraically. With
  lp0 + lp1 summed in log space, exp'd, mixed by w_sum, then logsumexp'd,
the whole network is
  out[b] = log( sum_{k,m} w_sum[k] * W0[k,m,x0_b] * W1[k,m,x1_b] )
        = log( A[x0_b, x1_b] ),   A = sum_k w_k * W0[k].T @ W1[k]  (shape [C, C])

Distribution: shard the latent axis k (256) across 8 cores (32 each). Each
core reads only its W shard, quantized to fp8e4m3 on host (w_sum and a
power-of-two range scale folded in), computes its partial
A_c = sum_{km} w0q[km,:]^T w1q[km,:] with DoubleRow fp8 matmuls (two
128-row chunks contracted per instruction), and DMAs the [256,256] bf16
partial back. The host sums the 8 partials (undoing each core's scale)
and evaluates log A at the 1024 (x0_b, x1_b) index pairs.

The program is RAW bass (no TileContext): hand-wired semaphores let the
SP queue start streaming weight pieces ~5 us earlier than the tile
framework's all-engine entry barrier would, so the DMA (the roofline
resource: 2 x 2.1 MB fp8 per core) overlaps the PE's slow ucode boot.
The 64 DoubleRow matmuls chase the pieces; DVE drains each PSUM half as
soon as its accumulation group stops, and the partials stream out on the
Activation/SP queues.
"""

import sys
from contextlib import ExitStack

import numpy as np

sys.path.insert(0, "/opt/trn_rl_repo")

import ml_dtypes

B, V, M, C = 1024, 2, 256, 256
NCORES = 8
KSH = M // NCORES          # k per core = 32
KM = KSH * M               # flattened contraction rows per core = 8192
NCHUNK = KM // 128         # 64 contraction chunks of 128 rows
NPAIR = NCHUNK // 2        # 32 DoubleRow chunk pairs

# DMA pieces, in chunk-pairs (sums to 32)
PIECES = [2, 2, 4, 4, 4, 4, 4, 4, 2, 1, 1]
assert sum(PIECES) == NPAIR

_cache = {}


def _build_program():
    import concourse.bacc as bacc
    import concourse.mybir as mybir

    f32 = mybir.dt.float32
    bf16 = mybir.dt.bfloat16
    fp8 = mybir.dt.float8e4

    nc = bacc.Bacc("TRN2", target_bir_lowering=False, enable_partition_id=False)

    # one DRAM tensor PER PIECE so every weight DMA reads a fully
    # contiguous block (strided 2KB reads from a single [128, 16K] tensor
    # wreck HBM page efficiency: measured 277 GB/s vs 356 for sequential).
    # x0 layout within a piece: [pair i][half h][sub j][128 cols];
    # x1: [chunk j][256 cols]
    x0w = [
        nc.dram_tensor(f"x0w{q}", [128, n * 2 * C], fp8, kind="ExternalInput")
        for q, n in enumerate(PIECES)
    ]
    x1w = [
        nc.dram_tensor(f"x1w{q}", [128, n * 2 * C], fp8, kind="ExternalInput")
        for q, n in enumerate(PIECES)
    ]
    gout = nc.dram_tensor("gout", [128, 2 * C], bf16, kind="ExternalOutput")

    with ExitStack() as ctx:
        ecm = ctx.enter_context
        x0sb = ecm(nc.sbuf_tensor("x0sb", [128, NPAIR, 2, 2 * 128], fp8))
        x1sb = ecm(nc.sbuf_tensor("x1sb", [128, NPAIR, 2 * C], fp8))
        gsb = ecm(nc.sbuf_tensor("gsb", [128, 2 * C], bf16))
        a0 = ecm(nc.psum_tensor("a0", [128, C], f32))
        a1 = ecm(nc.psum_tensor("a1", [128, C], f32))
        a_ps = [a0, a1]
        # one sem per piece: its two DMAs (x0+x1) each land +16 from the 16
        # SDMA engines, so >=32 means the whole piece is resident
        sp = [ecm(nc.semaphore(f"sp{q}")) for q in range(len(PIECES))]
        smm = [ecm(nc.semaphore(f"smm{h}")) for h in range(2)]
        scp = ecm(nc.semaphore("scp"))
        scp2 = ecm(nc.semaphore("scp2"))
        sout = ecm(nc.semaphore("sout"))

        with nc.Block(no_gpsimd_drain=True) as block:

            @block.sync
            def _(sync):
                pos = 0
                for q, n in enumerate(PIECES):
                    sync.dma_start(
                        out=x0sb[:, pos : pos + n, :, :], in_=x0w[q][:, :]
                    ).then_inc(sp[q], 16)
                    pos += n
                # h=1 half (the critical one: its matmul finishes last);
                # its cast lands on DVE, signalled via scp
                sync.wait_ge(scp, 1)
                sync.dma_start(
                    out=gout[:, C : 2 * C], in_=gsb[:, C : 2 * C]
                ).then_inc(sout, 16)
                sync.wait_ge(sout, 32)

            @block.scalar
            def _(scalar):
                pos = 0
                for q, n in enumerate(PIECES):
                    scalar.dma_start(
                        out=x1sb[:, pos : pos + n, :], in_=x1w[q][:, :]
                    ).then_inc(sp[q], 16)
                    pos += n
                # drain the h=0 PSUM half (its accumulation stops one matmul
                # earlier) on the activation engine; the extra sem hop to its
                # own out-DMA hides under the h=1 critical chain
                scalar.wait_ge(smm[0], 1)
                scalar.copy(gsb[:, 0:C], a0[:, :]).then_inc(scp2, 1)
                scalar.wait_ge(scp2, 1)
                scalar.dma_start(
                    out=gout[:, 0:C], in_=gsb[:, 0:C]
                ).then_inc(sout, 16)

            @block.tensor
            def _(tensor):
                pos = 0
                for q, n in enumerate(PIECES):
                    tensor.wait_ge(sp[q], 32)
                    for h in range(2):
                        for i in range(pos, pos + n):
                            mm = tensor.matmul(
                                a_ps[h][:, :],
                                lhsT=x0sb[:, i, h, :].rearrange(
                                    "p (two f) -> p two f", two=2
                                ),
                                rhs=x1sb[:, i, :].rearrange(
                                    "p (two f) -> p two f", two=2
                                ),
                                start=(i == 0),
                                stop=(i == NPAIR - 1),
                                perf_mode=mybir.MatmulPerfMode.DoubleRow,
                            )
                            if i == NPAIR - 1:
                                mm.then_inc(smm[h], 1)
                    pos += n

            @block.vector
            def _(vector):
                vector.wait_ge(smm[1], 1)
                vector.tensor_copy(gsb[:, C : 2 * C], a1[:, :]).then_inc(scp, 1)

    nc.compile()
    return nc


def _prep_inputs(x, W, w_sum):
    fp8 = ml_dtypes.float8_e4m3
    x = np.asarray(x)
    W = np.asarray(W, dtype=np.float32)
    w_sum = np.asarray(w_sum, dtype=np.float32)

    in_maps = []
    scales = []
    for c in range(NCORES):
        k0 = c * KSH
        w0 = (W[0, k0 : k0 + KSH] * w_sum[k0 : k0 + KSH, None, None]).reshape(KM, C)
        w1 = W[1, k0 : k0 + KSH].reshape(KM, C)
        # power-of-two scales put each shard's max near 128 (safe for any
        # e4m3 flavor) without adding rounding error of their own
        s0 = 2.0 ** np.floor(np.log2(128.0 / w0.max()))
        s1 = 2.0 ** np.floor(np.log2(128.0 / w1.max()))
        q0 = (w0 * s0).astype(fp8).reshape(NPAIR, 2, 128, 2, 128)
        # [pair i, sub j, part p, half h, col] -> [p, i, h, j, col]
        x0wc = q0.transpose(2, 0, 3, 1, 4).reshape(128, NCHUNK * C)
        q1 = (w1 * s1).astype(fp8).reshape(NCHUNK, 128, C)
        x1wc = q1.transpose(1, 0, 2).reshape(128, NCHUNK * C)
        im = {}
        pos = 0
        for q, n in enumerate(PIECES):
            fs = slice(pos * 2 * C, (pos + n) * 2 * C)
            im[f"x0w{q}"] = np.ascontiguousarray(x0wc[:, fs])
            im[f"x1w{q}"] = np.ascontiguousarray(x1wc[:, fs])
            pos += n
        in_maps.append(im)
        scales.append(1.0 / (float(s0) * float(s1)))
    return in_maps, scales


def _run(in_maps, **kwargs):
    from concourse.bass_utils import run_bass_kernel_spmd

    if "nc" not in _cache:
        _cache["nc"] = _build_program()
    return run_bass_kernel_spmd(
        _cache["nc"], in_maps, core_ids=list(range(NCORES)), **kwargs
    )


def _unshard(results, scales, x):
    x = np.asarray(x)
    A = np.zeros((C, C), dtype=np.float64)
    for r, inv_s in zip(results, scales):
        # gout[p, h*C + c] = A_c[h*128 + p, c]
        Ac = r["gout"].astype(np.float64).reshape(128, 2, C).transpose(1, 0, 2)
        A += Ac.reshape(C, C) * inv_s
    vals = A[x[:, 0].astype(np.int64), x[:, 1].astype(np.int64)]
    return np.log(vals).astype(np.float32)


def kernel(x, W, w_sum):
    in_maps, scales = _prep_inputs(x, W, w_sum)
    res = _run(in_maps)
    return _unshard(res.results, scales, x)

